# revision 47
# baseline (speedup 1.0000x reference)
"""DCGRU cell (nn_DCGRUCell) Trainium2 Bass kernel, 8 NeuronCores.

Sharding: node dimension N=4096 split 8 ways (512 rows/core); supports are
fed host-transposed (T = A^T), held resident in SBUF as bf16.

The bc feature columns are reordered host-side into [1024 state columns
(batch-major); 32 x columns]. Diffusion is linear per column and the
candidate shares its x part with the GCN-1 input, so the x-column
diffusions (A@x and A@A@x per support) are computed ONCE and reused by
both GCNs; the main hop sweeps handle exactly 1024 batch-aligned state
columns. Hop-1 products are computed node-major and AllGathered in two
512-column chunks per GCN (chunk A fires after the first sweep); hop-2
products are computed directly in transposed (feature-major) form. All
matmul operands are bf16 (PSUM fp32); the Chebyshev x2 = 2*A@x1 - x0 is
folded into the dense W host-side, and the 330-feature dense contraction
runs as 3 dense K-tiles of 110. The dense stage processes batches 4 at a
time, interleaved with the hop-2 passes, and the candidate state columns
are gathered in four batch-aligned 256-column chunks fired as soon as
each is staged. DMA issue is split across the two HWDGE rings (scalar:
GCN1/support loads + stores; sync: AG-gated loads).

kernel(**inputs) takes the FULL inputs from reference.setup_inputs() and
returns the FULL [16, 4096, 64] float32 output.
"""
import os
import numpy as np

import concourse.bass as bass
import concourse.mybir as mybir
import concourse.tile as tile
from concourse import bacc
from concourse.bass_utils import run_bass_kernel_spmd

F32 = mybir.dt.float32
BF16 = mybir.dt.bfloat16
AF = mybir.ActivationFunctionType

NCORES = 8
B, N, H, DIN = 16, 4096, 64, 2
C = DIN + H                 # 66 features per batch into each GCN
BC = B * C                  # 1056
SN = B * H                  # 1024 state columns
XC = B * DIN                # 32 x columns
NOWN = N // NCORES          # 512 rows per core
NT = NOWN // 128            # 4 n-tiles per core
MT = N // 128               # 32 m-tiles (contraction)
NQ = MT // NT               # 8 rank-blocks of 4 m-tiles
NB = 4                      # batches per W-stage iteration
CS = NB * H                 # 256 state columns per candidate chunk
KT = 110                    # dense-W K-tile height (330 = 3*110)
GROUP = [list(range(NCORES))]

_NC_CACHE = {}


def build_nc():
    nc = bacc.Bacc("TRN2", target_bir_lowering=False, debug=False,
                   num_devices=NCORES)

    d = {}
    d["Ts"] = nc.dram_tensor("Ts", [2, N, NOWN], BF16, kind="ExternalInput")
    d["st_nm"] = nc.dram_tensor("st_nm", [N, SN], BF16,
                                kind="ExternalInput")
    d["x_nm"] = nc.dram_tensor("x_nm", [MT, 128, XC], BF16,
                               kind="ExternalInput")
    d["xsT_own"] = nc.dram_tensor("xsT_own", [BC, NOWN], BF16,
                                  kind="ExternalInput")
    d["Wg"] = nc.dram_tensor("Wg", [5 * C, 2 * H], BF16, kind="ExternalInput")
    d["bg"] = nc.dram_tensor("bg", [2 * H, 1], F32, kind="ExternalInput")
    d["Wu"] = nc.dram_tensor("Wu", [5 * C, H], BF16, kind="ExternalInput")
    d["bu"] = nc.dram_tensor("bu", [H, 1], F32, kind="ExternalInput")
    d["outT"] = nc.dram_tensor("outT", [B, H, NOWN], F32,
                               kind="ExternalOutput")

    with tile.TileContext(nc) as tc:
        _emit(nc, tc, d)
    nc.compile()
    return nc


def _emit(nc, tc, d):
    import contextlib
    stack = contextlib.ExitStack()
    with stack:
        const = stack.enter_context(tc.tile_pool(name="const", bufs=1))
        sb_ex = stack.enter_context(tc.tile_pool(name="ex", bufs=1))
        sb_mov = stack.enter_context(tc.tile_pool(name="mov", bufs=1))
        sb_sm = stack.enter_context(tc.tile_pool(name="small", bufs=1))
        dram = stack.enter_context(
            tc.tile_pool(name="dram", bufs=1, space="DRAM"))
        psum = stack.enter_context(
            tc.tile_pool(name="psum", bufs=1, space="PSUM"))

        # ---- resident support tiles (loaded staggered in first sweep) ----
        Tch = {}
        for s in range(2):
            for k in range(NQ):
                Tch[(s, k)] = const.tile([128, NT, 512], BF16,
                                         name=f"T{s}_{k}")

        def load_Tch(k):
            for s in range(2):
                ts = d["Ts"].ap()[s].rearrange("(t p) n -> p t n", p=128)
                nc.scalar.dma_start(Tch[(s, k)][:],
                                    ts[:, k * NT:(k + 1) * NT, :])

        load_Tch(0)
        load_Tch(1)

        def T_tile(s, m):
            return Tch[(s, m // NT)][:, m % NT, :]

        ident = const.tile([128, 128], F32)
        nc.gpsimd.memset(ident[:], 0.0)
        nc.gpsimd.affine_select(
            out=ident[:], in_=ident[:],
            compare_op=mybir.AluOpType.not_equal, fill=1.0, base=0,
            pattern=[[-1, 128]], channel_multiplier=1)
        identb = const.tile([128, 128], BF16)
        nc.vector.tensor_copy(identb[:], ident[:])

        # dense-W constants: K-tiled [3, 110, out] (DMAs emitted later so
        # the scalar ring stays clear at startup)
        wg_t = const.tile([KT, 3, 2 * H], BF16)
        wu_t = const.tile([KT, 3, H], BF16)
        bg_t = const.tile([2 * H, 1], F32)
        bu_t = const.tile([H, 1], F32)

        def load_w_consts():
            for k in range(3):
                nc.scalar.dma_start(wg_t[:, k, :],
                                    d["Wg"].ap()[k * KT:(k + 1) * KT, :])
                nc.scalar.dma_start(wu_t[:, k, :],
                                    d["Wu"].ap()[k * KT:(k + 1) * KT, :])
            nc.scalar.dma_start(bg_t[:], d["bg"].ap())
            nc.scalar.dma_start(bu_t[:], d["bu"].ap())

        # row-run map for the dense stage: K-tile k's partition range
        # [off, off+w) reads block j (0 = direct input, 1-4 = diffusion
        # outputs y1_s0, y2_s0, y1_s1, y2_s1), feature cols [c, c+w);
        # within each block features are [state(64); x(2)] per batch
        KT_RUNS = []
        r0 = 0
        while r0 < 330:
            k, off = divmod(r0, KT)
            j, c = divmod(r0, C)
            w = min(C - c, KT - off)
            KT_RUNS.append((k, off, j, c, w))
            r0 += w

        # ---- DRAM staging ----
        agA = [dram.tile([2 * NT * 128 * 512], BF16, name=f"agA{g}")
               for g in range(2)]
        agB = [[dram.tile([NT * 128 * 512], BF16, name=f"agB{g}{s}")
                for s in range(2)] for g in range(2)]
        agAo = [dram.tile([NCORES * 2 * NT * 128 * 512], BF16,
                          name=f"agAo{g}", addr_space="Shared")
                for g in range(2)]
        agBo = [[dram.tile([NCORES * NT * 128 * 512], BF16,
                           name=f"agBo{g}{s}", addr_space="Shared")
                 for s in range(2)] for g in range(2)]
        # x-column diffusion (shared by both GCNs): node-major y1x + gather
        agX = dram.tile([2 * NT * 128 * XC], BF16, name="agX")
        agXo = dram.tile([NCORES * 2 * NT * 128 * XC], BF16, name="agXo",
                         addr_space="Shared")
        # candidate state columns: chunks of 256/256/512 (the last two
        # stage groups share one gather)
        CW = [CS, CS, 2 * CS]
        candC = [dram.tile([NT * 128 * CW[i]], BF16, name=f"candC{i}")
                 for i in range(3)]
        candCo = [dram.tile([NCORES * NT * 128 * CW[i]], BF16,
                            name=f"candCo{i}", addr_space="Shared")
                  for i in range(3)]
        # feature-major staging: y1 state rows whole, y2 state rows split
        # at 512 (batch 8); x-diffusion rows in their own small tiles
        y1t = [[dram.tile([SN, NOWN], BF16, name=f"y1_{g}{s}")
                for s in range(2)] for g in range(2)]
        y2tA = [[dram.tile([512, NOWN], BF16, name=f"y2A_{g}{s}")
                 for s in range(2)] for g in range(2)]
        y2tB = [[dram.tile([512, NOWN], BF16, name=f"y2B_{g}{s}")
                 for s in range(2)] for g in range(2)]
        y1xT = [dram.tile([XC, NOWN], BF16, name=f"y1x{s}")
                for s in range(2)]
        y2xT = [dram.tile([XC, NOWN], BF16, name=f"y2x{s}")
                for s in range(2)]
        candT_dram = dram.tile([BC, NOWN], BF16)
        rt_dram = dram.tile([B, H, NOWN], BF16)

        def agA_own(g, s, t):
            o = ((s * NT + t) * 128) * 512
            return agA[g].opt()[o:o + 128 * 512].rearrange(
                "(p f) -> p f", f=512)

        def agB_own(g, s, t):
            o = (t * 128) * 512
            return agB[g][s].opt()[o:o + 128 * 512].rearrange(
                "(p f) -> p f", f=512)

        def agX_own(s, t):
            o = ((s * NT + t) * 128) * XC
            return agX.opt()[o:o + 128 * XC].rearrange(
                "(p f) -> p f", f=XC)

        def outA_q(g, s, q):
            o = ((q * 2 + s) * NT * 128) * 512
            return agAo[g].opt()[o:o + NT * 128 * 512].rearrange(
                "(t p f) -> p t f", p=128, f=512)

        def outB_q(g, s, q):
            o = (q * NT * 128) * 512
            return agBo[g][s].opt()[o:o + NT * 128 * 512].rearrange(
                "(t p f) -> p t f", p=128, f=512)

        def outX_s(s):
            v = agXo.opt().rearrange("(q s2 t p f) -> s2 p q t f",
                                     s2=2, t=NT, p=128, f=XC)
            return v[s]

        def candC_own(i, t):
            o = t * 128 * CW[i]
            return candC[i].opt()[o:o + 128 * CW[i]].rearrange(
                "(p f) -> p f", f=CW[i])

        def candCo_q(i, q):
            o = q * NT * 128 * CW[i]
            return candCo[i].opt()[o:o + NT * 128 * CW[i]].rearrange(
                "(t p f) -> p t f", p=128, f=CW[i])

        def allgather(src, dst):
            nc.gpsimd.collective_compute(
                "AllGather", mybir.AluOpType.bypass, replica_groups=GROUP,
                ins=[src.opt()], outs=[dst.opt()])

        # ========== x-column diffusion hop 1 (once, both supports) =======
        def make_xpass():
            """y1x_s = A_s @ x: feature-major direct + node-major for the
            gather. Loads fire mid-first-sweep; the MMs run between the
            first main sweep and its staging (PSUM tiles allocated there
            so slot rotation can't deadlock against the sweep)."""
            pre = {}

            def loads():
                for q in range(NQ):
                    mx = sb_mov.tile([128, NT, XC], BF16, name=f"mx{q}",
                                     tag="movr", bufs=8)
                    src = d["x_nm"].ap().rearrange("(q t) p f -> q p t f",
                                                   t=NT)
                    nc.scalar.dma_start(mx[:], src[q])
                    pre[q] = mx

            ps_x = [None, None]

            def mms():
                for s in range(2):
                    ps_x[s] = psum.tile([XC, NOWN], F32, name=f"psx{s}",
                                        tag="acc", bufs=8)
                for q in range(NQ):
                    for tt in range(NT):
                        m = q * NT + tt
                        for s in range(2):
                            nc.tensor.matmul(ps_x[s][:], pre[q][:, tt, :],
                                             T_tile(s, m), start=(m == 0),
                                             stop=(m == MT - 1))

            def stage():
                for s in range(2):
                    xe = sb_ex.tile([XC, NOWN], BF16, name=f"xex{s}",
                                    tag="ragex", bufs=4)
                    nc.vector.tensor_copy(xe[:], ps_x[s][:])
                    nc.scalar.dma_start(y1xT[s].opt()[:, :], xe[:])
                    for t in range(NT):
                        tp = psum.tile([128, XC], BF16, name=f"xtp{s}",
                                       tag="acc", bufs=8)
                        nc.tensor.transpose(
                            tp[:], xe[:, t * 128:(t + 1) * 128],
                            identb[0:XC, 0:XC])
                        xnm = sb_sm.tile([128, XC], BF16, name=f"xnm{s}",
                                         tag="rnm", bufs=8)
                        nc.vector.tensor_copy(xnm[:], tp[:])
                        nc.scalar.dma_start(agX_own(s, t), xnm[:])
                allgather(agX, agXo)
            return loads, mms, stage

        # ========== x-column diffusion hop 2 (once, both supports) =======
        def emit_x2pass():
            for s in range(2):
                ps2 = psum.tile([XC, NOWN], F32, name=f"psx2{s}",
                                tag="acc", bufs=8)
                mrX = sb_mov.tile([128, NT, NQ, XC], BF16, name=f"mrX{s}",
                                  tag="mrX", bufs=2)
                vX = outX_s(s)
                for t in range(NT):
                    nc.sync.dma_start(mrX[:, t, :, :], vX[:, :, t, :])
                for q in range(NQ):
                    for tt in range(NT):
                        m = q * NT + tt
                        nc.tensor.matmul(ps2[:], mrX[:, tt, q, :],
                                         T_tile(s, m), start=(m == 0),
                                         stop=(m == MT - 1))
                xe = sb_ex.tile([XC, NOWN], BF16, name=f"x2ex{s}",
                                tag="ragex", bufs=4)
                nc.vector.tensor_copy(xe[:], ps2[:])
                nc.scalar.dma_start(y2xT[s].opt()[:, :], xe[:])

        # ============ hop-1 state sweeps + overlapped AG ============
        def emit_hop1_pair(pid, g, load_mov, y1_dst, stagger_T, xpass):
            """Y1_s[own rows, state cols] = A_s @ M for s in (0, 1). AG
            chunk A fires after the first sweep, B after the second.
            Feature-major y1 transposes are deferred (returned closure)."""
            kept = {}
            preloaded = {}
            xloads, xmms, xstage = xpass if xpass else (None, None, None)

            def emit_dyt(hh):
                # feature-major y1 staging for one sweep's outputs; h0's
                # runs between the sweeps (fills the AG-A window), h1's is
                # interleaved by the caller between the hop-2 A passes
                for s in range(2):
                    for j2 in range(2):
                        st4 = sb_sm.tile([128, 2, NOWN], BF16,
                                         name=f"st4{pid}", tag="st",
                                         bufs=4)
                        for jj2 in range(2):
                            j = j2 * 2 + jj2
                            for n in range(NT):
                                tp = psum.tile([128, 128], BF16,
                                               name=f"tp{pid}",
                                               tag="acc", bufs=8)
                                nc.tensor.transpose(
                                    tp[:],
                                    kept[(hh, s)][n][
                                        :, j * 128:(j + 1) * 128],
                                    identb[:])
                                nc.vector.tensor_copy(
                                    st4[:, jj2,
                                        n * 128:(n + 1) * 128], tp[:])
                        jj = hh * 4 + j2 * 2
                        nc.scalar.dma_start(
                            y1_dst[s].opt()[jj * 128:(jj + 2) * 128, :]
                            .rearrange("(j p) n -> p j n", j=2),
                            st4[:])
            for hh in range(2):
                ps_m = {}
                for s in range(2):
                    for n in range(NT):
                        ps_m[(s, n)] = psum.tile(
                            [128, 512], F32, name=f"psm{pid}_{hh}{s}{n}",
                            tag="acc", bufs=8)
                for q in range(NQ):
                    if stagger_T and hh == 0 and q + 2 < NQ:
                        load_Tch(q + 2)
                    if hh == 0 and q == 2 and xloads is not None:
                        xloads()
                    if (hh, q) in preloaded:
                        mv4 = preloaded.pop((hh, q))
                    else:
                        mv4 = sb_mov.tile([128, NT, 512], BF16,
                                          name=f"mv{pid}_{hh}_{q}",
                                          tag="mov", bufs=5)
                        load_mov(mv4, q, hh)
                    for tt in range(NT):
                        m = q * NT + tt
                        for s in range(2):
                            for n in range(NT):
                                nc.tensor.matmul(
                                    ps_m[(s, n)][:],
                                    T_tile(s, m)[:, n * 128:(n + 1) * 128],
                                    mv4[:, tt, :], start=(m == 0),
                                    stop=(m == MT - 1))
                if hh == 0:
                    # prefetch the next sweep's first tiles ahead of the
                    # staging stores
                    for qq in range(2):
                        mv4p = sb_mov.tile([128, NT, 512], BF16,
                                           name=f"mv{pid}_1_{qq}",
                                           tag="mov", bufs=5)
                        load_mov(mv4p, qq, 1)
                        preloaded[(1, qq)] = mv4p
                    # x-column matmuls bridge the PE across the AG-A
                    # window (their PSUM slots free after the first two
                    # extracts)
                    if xmms is not None:
                        xmms()
                for s in range(2):
                    exhs = []
                    for n in range(NT):
                        exh = sb_ex.tile([128, 512], BF16,
                                         name=f"ex{pid}{hh}{s}{n}",
                                         tag="ex", bufs=16)
                        nc.vector.tensor_copy(exh[:], ps_m[(s, n)][:])
                        dst = (agA_own(g, s, n) if hh == 0
                               else agB_own(g, s, n))
                        nc.scalar.dma_start(dst, exh[:])
                        exhs.append(exh)
                    kept[(hh, s)] = exhs
                    if hh == 1:
                        allgather(agB[g][s], agBo[g][s])
                if hh == 0:
                    allgather(agA[g], agAo[g])
                if hh == 0:
                    if xstage is not None:
                        xstage()
                    emit_dyt(0)

            return lambda: emit_dyt(1)

        # ======= hop-2 product: transposed form (feature-major out) =======
        def emit_hop2_pass(pid, s, g, part, yA_dst, yB_dst):
            ps = [psum.tile([128, NOWN], F32, name=f"ph2{pid}_{j}",
                            tag="acc", bufs=8) for j in range(4)]
            for q in range(NQ):
                mr4 = sb_mov.tile([128, NT, 512], BF16,
                                  name=f"mr{pid}_{q}", tag="mov", bufs=5)
                nc.sync.dma_start(
                    mr4[:], outA_q(g, s, q) if part == 0
                    else outB_q(g, s, q))
                for tt in range(NT):
                    m = q * NT + tt
                    for j in range(4):
                        nc.tensor.matmul(
                            ps[j][:],
                            mr4[:, tt, j * 128:(j + 1) * 128],
                            T_tile(s, m), start=(m == 0),
                            stop=(m == MT - 1))
            dst_t = yA_dst if part == 0 else yB_dst
            for j in range(4):
                exh = sb_ex.tile([128, NOWN], BF16, name=f"h2ex{pid}_{j}",
                                 tag="ex", bufs=16)
                nc.vector.tensor_copy(exh[:], ps[j][:])
                nc.scalar.dma_start(
                    dst_t.opt()[j * 128:(j + 1) * 128, :], exh[:])

        # ============== dense W stage (K-tiled, 4 batches/iter) ==========
        def load_ktiles(pi, block0_src, g):
            """Assemble the 330-feature contraction as 3 dense K-tiles of
            110 partitions. Block features are [state(64); x(2)] per
            batch: state rows come from the y1/y2 state tiles (y2 split
            A/B at batch 8, always batch-aligned), x rows from the shared
            x-diffusion tiles."""
            b0 = NB * pi
            kts = [sb_sm.tile([KT, NB, NOWN], BF16, name=f"kt{k}",
                              tag=f"kt{k}", bufs=3) for k in range(3)]
            for k, off, j, c, w in KT_RUNS:
                if j == 0:
                    nc.sync.dma_start(
                        kts[k][off:off + w, :, :],
                        block0_src[b0 * C:(b0 + NB) * C, :]
                        .rearrange("(b c) n -> c b n", b=NB)[c:c + w])
                    continue
                sidx = (j - 1) // 2 if j % 2 == 1 else j // 2 - 1
                # state part
                if c < H:
                    cs_hi = min(c + w, H)
                    if j % 2 == 1:
                        src = y1t[g][sidx].opt()
                        rlo = b0 * H
                    else:
                        src = (y2tA[g][sidx] if pi < 2
                               else y2tB[g][sidx]).opt()
                        rlo = b0 * H - (0 if pi < 2 else 512)
                    nc.sync.dma_start(
                        kts[k][off:off + cs_hi - c, :, :],
                        src[rlo:rlo + NB * H, :]
                        .rearrange("(b c) n -> c b n", b=NB)[c:cs_hi])
                # x part
                if c + w > H:
                    xsrc = (y1xT if j % 2 == 1 else y2xT)[sidx]
                    nc.sync.dma_start(
                        kts[k][off + max(0, H - c):off + w, :, :],
                        xsrc.opt()[b0 * DIN:(b0 + NB) * DIN, :]
                        .rearrange("(b c) n -> c b n",
                                   b=NB)[max(0, c - H):c + w - H])
            return kts

        def gate_mm(pi):
            b0 = NB * pi
            kts = load_ktiles(pi, d["xsT_own"].ap(), 0)
            zr_ps = [psum.tile([2 * H, NOWN], F32, name=f"zrps{b2}",
                               tag="acc", bufs=8) for b2 in range(NB)]
            for k in range(3):
                for b2 in range(NB):
                    nc.tensor.matmul(zr_ps[b2][:], wg_t[:, k, :],
                                     kts[k][:, b2, :],
                                     start=(k == 0), stop=(k == 2))
            zr4 = sb_sm.tile([2 * H, NB, NOWN], BF16, name="zr", tag="zr",
                             bufs=2)
            for b2 in range(NB):
                nc.scalar.activation(zr4[:, b2, :], zr_ps[b2][:],
                                     AF.Sigmoid, bias=bg_t[:])
            nc.scalar.dma_start(
                rt_dram.opt()[b0:b0 + NB].rearrange("b (h n) -> h b n",
                                                    h=H),
                zr4[H:2 * H, :, :])
            # kts[0][0:C] holds the direct input rows [state(64); x(2)]
            cT4 = sb_sm.tile([C, NB, NOWN], BF16, name="cT", tag="cT",
                             bufs=2)
            nc.vector.tensor_mul(cT4[0:H, :, :], zr4[0:H, :, :],
                                 kts[0][0:H, :, :])
            nc.vector.tensor_copy(cT4[H:C, :, :], kts[0][H:C, :, :])
            nc.scalar.dma_start(
                candT_dram.opt()[b0 * C:(b0 + NB) * C, :]
                .rearrange("(b c) n -> c b n", b=NB), cT4[:])
            return cT4

        def cand_stage(pi, cT4):
            # node-major candidate state chunk (batch-aligned, 256 cols);
            # chunks 2 and 3 share a buffer and gather once
            ci = min(pi, 2)
            co = (pi - 2) * CS if pi > 2 else 0
            for t in range(NT):
                ct4 = sb_sm.tile([128, NB, H], BF16, name="ctnm",
                                 tag="ctnm", bufs=4)
                for b2 in range(NB):
                    tp = psum.tile([128, H], BF16, name="ctps", tag="acc",
                                   bufs=8)
                    nc.tensor.transpose(
                        tp[:], cT4[0:H, b2, t * 128:(t + 1) * 128],
                        identb[0:H, 0:H])
                    nc.vector.tensor_copy(ct4[:, b2, :], tp[:])
                nc.scalar.dma_start(
                    candC_own(ci, t)[:, co:co + CS],
                    ct4[:].rearrange("p b c -> p (b c)"))
            if pi != 2:
                allgather(candC[ci], candCo[ci])

        def update_pi(pi):
            b0 = NB * pi
            kts = load_ktiles(pi, candT_dram.opt(), 1)
            hc_ps = [psum.tile([H, NOWN], F32, name=f"hcps{b2}", tag="acc",
                               bufs=8) for b2 in range(NB)]
            for k in range(3):
                for b2 in range(NB):
                    nc.tensor.matmul(hc_ps[b2][:], wu_t[:, k, :],
                                     kts[k][:, b2, :],
                                     start=(k == 0), stop=(k == 2))
            # out = hc + r * (state - hc); per-batch pipelined so the
            # trailing activation/DVE/store chain after the last matmul is
            # one batch deep, not four
            stT4 = sb_sm.tile([H, NB, NOWN], BF16, name="stTu", tag="stg",
                              bufs=2)
            nc.sync.dma_start(
                stT4[:],
                d["xsT_own"].ap()[b0 * C:(b0 + NB) * C, :]
                .rearrange("(b c) n -> c b n", b=NB)[0:H])
            rT4 = sb_sm.tile([H, NB, NOWN], BF16, name="rT", tag="rT",
                             bufs=2)
            nc.sync.dma_start(
                rT4[:],
                rt_dram.opt()[b0:b0 + NB].rearrange("b (h n) -> h b n",
                                                    h=H))
            hc4 = sb_sm.tile([H, NB, NOWN], BF16, name="hc", tag="zr",
                             bufs=2)
            tmp4 = sb_sm.tile([H, NB, NOWN], BF16, name="tmp", tag="tmp",
                              bufs=1)
            ot4 = sb_sm.tile([H, NB, NOWN], F32, name="ot", tag="ot",
                             bufs=1)
            for b2 in range(NB):
                nc.scalar.activation(hc4[:, b2, :], hc_ps[b2][:], AF.Tanh,
                                     bias=bu_t[:])
                nc.vector.tensor_sub(tmp4[:, b2, :], stT4[:, b2, :],
                                     hc4[:, b2, :])
                nc.vector.tensor_mul(tmp4[:, b2, :], rT4[:, b2, :],
                                     tmp4[:, b2, :])
                nc.vector.tensor_add(ot4[:, b2, :], hc4[:, b2, :],
                                     tmp4[:, b2, :])
                nc.scalar.dma_start(d["outT"].ap()[b0 + b2],
                                    ot4[:, b2, :])

        # ======================= GCN 1 (gate) =======================
        def g1_load_mov(t4, q, hh):
            src = d["st_nm"].ap().rearrange("(q t p) f -> q p t f",
                                            p=128, t=NT)
            nc.scalar.dma_start(t4[:],
                                src[q, :, :, hh * 512:(hh + 1) * 512])

        dyt1 = emit_hop1_pair("g1h1", 0, g1_load_mov, y1t[0], True,
                              make_xpass())
        dyt1()
        emit_hop2_pass("g1s0h2A", 0, 0, 0, y2tA[0][0], y2tB[0][0])
        emit_hop2_pass("g1s1h2A", 1, 0, 0, y2tA[0][1], y2tB[0][1])
        emit_x2pass()
        load_w_consts()
        ct0 = gate_mm(0)
        cand_stage(0, ct0)
        ct1 = gate_mm(1)
        cand_stage(1, ct1)
        emit_hop2_pass("g1s0h2B", 0, 0, 1, y2tA[0][0], y2tB[0][0])
        emit_hop2_pass("g1s1h2B", 1, 0, 1, y2tA[0][1], y2tB[0][1])
        ct2 = gate_mm(2)
        ct3 = gate_mm(3)
        cand_stage(2, ct2)
        cand_stage(3, ct3)

        # ======================= GCN 2 (update) =======================
        def g2_load_mov(t4, q, hh):
            if hh == 0:
                nc.sync.dma_start(t4[:, :, 0:CS], candCo_q(0, q))
                nc.sync.dma_start(t4[:, :, CS:512], candCo_q(1, q))
            else:
                nc.sync.dma_start(t4[:], candCo_q(2, q))

        dyt2 = emit_hop1_pair("g2h1", 1, g2_load_mov, y1t[1], False, None)
        dyt2()
        emit_hop2_pass("g2s0h2A", 0, 1, 0, y2tA[1][0], y2tB[1][0])
        emit_hop2_pass("g2s1h2A", 1, 1, 0, y2tA[1][1], y2tB[1][1])
        update_pi(0)
        update_pi(1)
        emit_hop2_pass("g2s0h2B", 0, 1, 1, y2tA[1][0], y2tB[1][0])
        emit_hop2_pass("g2s1h2B", 1, 1, 1, y2tA[1][1], y2tB[1][1])
        update_pi(2)
        update_pi(3)


def prepare_in_maps(x, state, support0, support1, W_gate, b_gate,
                    W_update, b_update):
    BFNP = mybir.dt.np(BF16)
    x_f = np.asarray(x, dtype=np.float32)
    state_f = np.asarray(state, dtype=np.float32)
    # node-major state (batch-major columns) and x columns
    st_nm = np.ascontiguousarray(
        state_f.transpose(1, 0, 2).reshape(N, SN)).astype(BFNP)
    x_nm = np.ascontiguousarray(
        x_f.transpose(1, 0, 2).reshape(N, XC)).astype(BFNP).reshape(
            MT, 128, XC)
    # feature-major input for W / elementwise uses [state(64); x(2)] rows
    sx_nm = np.ascontiguousarray(
        np.concatenate([state_f, x_f], axis=-1)
        .transpose(1, 0, 2).reshape(N, BC)).astype(np.float32)
    perm = np.r_[DIN:C, 0:DIN]                 # [x, state] -> [state, x]

    # fold the Chebyshev combination x2 = 2*A@x1 - x0 into W:
    # W0 -= (W2 + W4); W2 *= 2; W4 *= 2  (per 66-row block); then permute
    # every block's rows to the device feature order [state; x]
    def fold_perm(W):
        Wf = np.ascontiguousarray(W, dtype=np.float32).copy()
        Wf[0:C] -= Wf[2 * C:3 * C] + Wf[4 * C:5 * C]
        Wf[2 * C:3 * C] *= 2.0
        Wf[4 * C:5 * C] *= 2.0
        for j in range(5):
            Wf[j * C:(j + 1) * C] = Wf[j * C:(j + 1) * C][perm]
        return Wf.astype(BFNP)

    Wg_dev = fold_perm(W_gate)
    Wu_dev = fold_perm(W_update)

    bg = np.ascontiguousarray(b_gate, dtype=np.float32).reshape(2 * H, 1)
    bu = np.ascontiguousarray(b_update, dtype=np.float32).reshape(H, 1)
    s0b = np.asarray(support0, dtype=np.float32).astype(BFNP)
    s1b = np.asarray(support1, dtype=np.float32).astype(BFNP)

    in_maps = []
    for r in range(NCORES):
        n0 = r * NOWN
        in_maps.append({
            "Ts": np.ascontiguousarray(
                np.stack([s0b[n0:n0 + NOWN, :].T,
                          s1b[n0:n0 + NOWN, :].T])),
            "st_nm": st_nm,
            "x_nm": x_nm,
            "xsT_own": np.ascontiguousarray(
                sx_nm[n0:n0 + NOWN].T).astype(BFNP),
            "Wg": Wg_dev, "bg": bg, "Wu": Wu_dev, "bu": bu,
        })
    return in_maps


def assemble_output(results):
    out = np.empty((B, N, H), dtype=np.float32)
    for r in range(NCORES):
        n0 = r * NOWN
        out[:, n0:n0 + NOWN, :] = results[r]["outT"].transpose(0, 2, 1)
    return out


def get_nc():
    if "nc" not in _NC_CACHE:
        _NC_CACHE["nc"] = build_nc()
    return _NC_CACHE["nc"]


def kernel(x, state, support0, support1, W_gate, b_gate, W_update, b_update):
    nc = get_nc()
    in_maps = prepare_in_maps(x, state, support0, support1,
                              W_gate, b_gate, W_update, b_update)
    prev = os.environ.get("BASS_NEVER_TRACE")
    os.environ["BASS_NEVER_TRACE"] = "1"
    try:
        res = run_bass_kernel_spmd(nc, in_maps, list(range(NCORES)),
                                   trace=False)
    finally:
        if prev is None:
            os.environ.pop("BASS_NEVER_TRACE", None)
        else:
            os.environ["BASS_NEVER_TRACE"] = prev
    return assemble_output(res.results)


# revision 49
# speedup vs baseline: 1.0192x; 1.0192x over previous
"""DCGRU cell (nn_DCGRUCell) Trainium2 Bass kernel, 8 NeuronCores.

Sharding: node dimension N=4096 split 8 ways (512 rows/core); supports are
fed host-transposed (T = A^T), held resident in SBUF as bf16.

The bc feature columns are reordered host-side into [1024 state columns
(batch-major); 32 x columns]. Diffusion is linear per column and the
candidate shares its x part with the GCN-1 input, so the x-column
diffusions (A@x and A@A@x per support) are computed ONCE and reused by
both GCNs; the main hop sweeps handle exactly 1024 batch-aligned state
columns. Hop-1 products are computed node-major and AllGathered in two
512-column chunks per GCN (chunk A fires after the first sweep); hop-2
products are computed directly in transposed (feature-major) form. All
matmul operands are bf16 (PSUM fp32); the Chebyshev x2 = 2*A@x1 - x0 is
folded into the dense W host-side, and the 330-feature dense contraction
runs as 3 dense K-tiles of 110. The dense stage processes batches 4 at a
time, interleaved with the hop-2 passes, and the candidate state columns
are gathered in four batch-aligned 256-column chunks fired as soon as
each is staged. DMA issue is split across the two HWDGE rings (scalar:
GCN1/support loads + stores; sync: AG-gated loads).

kernel(**inputs) takes the FULL inputs from reference.setup_inputs() and
returns the FULL [16, 4096, 64] float32 output.
"""
import os
import numpy as np

import concourse.bass as bass
import concourse.mybir as mybir
import concourse.tile as tile
from concourse import bacc
from concourse.bass_utils import run_bass_kernel_spmd

F32 = mybir.dt.float32
BF16 = mybir.dt.bfloat16
AF = mybir.ActivationFunctionType

NCORES = 8
B, N, H, DIN = 16, 4096, 64, 2
C = DIN + H                 # 66 features per batch into each GCN
BC = B * C                  # 1056
SN = B * H                  # 1024 state columns
XC = B * DIN                # 32 x columns
NOWN = N // NCORES          # 512 rows per core
NT = NOWN // 128            # 4 n-tiles per core
MT = N // 128               # 32 m-tiles (contraction)
NQ = MT // NT               # 8 rank-blocks of 4 m-tiles
NB = 4                      # batches per W-stage iteration
CS = NB * H                 # 256 state columns per candidate chunk
KT = 110                    # dense-W K-tile height (330 = 3*110)
GROUP = [list(range(NCORES))]

_NC_CACHE = {}


def build_nc():
    nc = bacc.Bacc("TRN2", target_bir_lowering=False, debug=False,
                   num_devices=NCORES)

    d = {}
    d["Ts"] = nc.dram_tensor("Ts", [2, N, NOWN], BF16, kind="ExternalInput")
    d["st_nm"] = nc.dram_tensor("st_nm", [N, SN], BF16,
                                kind="ExternalInput")
    d["x_nm"] = nc.dram_tensor("x_nm", [MT, 128, XC], BF16,
                               kind="ExternalInput")
    d["xsT_own"] = nc.dram_tensor("xsT_own", [BC, NOWN], BF16,
                                  kind="ExternalInput")
    d["Wg"] = nc.dram_tensor("Wg", [5 * C, 2 * H], BF16, kind="ExternalInput")
    d["bg"] = nc.dram_tensor("bg", [2 * H, 1], F32, kind="ExternalInput")
    d["Wu"] = nc.dram_tensor("Wu", [5 * C, H], BF16, kind="ExternalInput")
    d["bu"] = nc.dram_tensor("bu", [H, 1], F32, kind="ExternalInput")
    d["outT"] = nc.dram_tensor("outT", [B, H, NOWN], F32,
                               kind="ExternalOutput")

    with tile.TileContext(nc) as tc:
        _emit(nc, tc, d)
    nc.compile()
    return nc


def _emit(nc, tc, d):
    import contextlib
    stack = contextlib.ExitStack()
    with stack:
        const = stack.enter_context(tc.tile_pool(name="const", bufs=1))
        sb_ex = stack.enter_context(tc.tile_pool(name="ex", bufs=1))
        sb_mov = stack.enter_context(tc.tile_pool(name="mov", bufs=1))
        sb_sm = stack.enter_context(tc.tile_pool(name="small", bufs=1))
        dram = stack.enter_context(
            tc.tile_pool(name="dram", bufs=1, space="DRAM"))
        psum = stack.enter_context(
            tc.tile_pool(name="psum", bufs=1, space="PSUM"))

        # ---- resident support tiles (loaded staggered in first sweep) ----
        Tch = {}
        for s in range(2):
            for k in range(NQ):
                Tch[(s, k)] = const.tile([128, NT, 512], BF16,
                                         name=f"T{s}_{k}")

        def load_Tch(k):
            for s in range(2):
                ts = d["Ts"].ap()[s].rearrange("(t p) n -> p t n", p=128)
                nc.scalar.dma_start(Tch[(s, k)][:],
                                    ts[:, k * NT:(k + 1) * NT, :])

        load_Tch(0)
        load_Tch(1)

        def T_tile(s, m):
            return Tch[(s, m // NT)][:, m % NT, :]

        ident = const.tile([128, 128], F32)
        nc.gpsimd.memset(ident[:], 0.0)
        nc.gpsimd.affine_select(
            out=ident[:], in_=ident[:],
            compare_op=mybir.AluOpType.not_equal, fill=1.0, base=0,
            pattern=[[-1, 128]], channel_multiplier=1)
        identb = const.tile([128, 128], BF16)
        nc.vector.tensor_copy(identb[:], ident[:])

        # dense-W constants: K-tiled [3, 110, out] (DMAs emitted later so
        # the scalar ring stays clear at startup)
        wg_t = const.tile([KT, 3, 2 * H], BF16)
        wu_t = const.tile([KT, 3, H], BF16)
        bg_t = const.tile([2 * H, 1], F32)
        bu_t = const.tile([H, 1], F32)

        def load_w_consts():
            for k in range(3):
                nc.scalar.dma_start(wg_t[:, k, :],
                                    d["Wg"].ap()[k * KT:(k + 1) * KT, :])
                nc.scalar.dma_start(wu_t[:, k, :],
                                    d["Wu"].ap()[k * KT:(k + 1) * KT, :])
            nc.scalar.dma_start(bg_t[:], d["bg"].ap())
            nc.scalar.dma_start(bu_t[:], d["bu"].ap())

        # row-run map for the dense stage: K-tile k's partition range
        # [off, off+w) reads block j (0 = direct input, 1-4 = diffusion
        # outputs y1_s0, y2_s0, y1_s1, y2_s1), feature cols [c, c+w);
        # within each block features are [state(64); x(2)] per batch
        KT_RUNS = []
        r0 = 0
        while r0 < 330:
            k, off = divmod(r0, KT)
            j, c = divmod(r0, C)
            w = min(C - c, KT - off)
            KT_RUNS.append((k, off, j, c, w))
            r0 += w

        # ---- DRAM staging ----
        agA = [dram.tile([2 * NT * 128 * 512], BF16, name=f"agA{g}")
               for g in range(2)]
        agB = [[dram.tile([NT * 128 * 512], BF16, name=f"agB{g}{s}")
                for s in range(2)] for g in range(2)]
        agAo = [dram.tile([NCORES * 2 * NT * 128 * 512], BF16,
                          name=f"agAo{g}", addr_space="Shared")
                for g in range(2)]
        agBo = [[dram.tile([NCORES * NT * 128 * 512], BF16,
                           name=f"agBo{g}{s}", addr_space="Shared")
                 for s in range(2)] for g in range(2)]
        # x-column diffusion (shared by both GCNs): node-major y1x + gather
        agX = dram.tile([2 * NT * 128 * XC], BF16, name="agX")
        agXo = dram.tile([NCORES * 2 * NT * 128 * XC], BF16, name="agXo",
                         addr_space="Shared")
        # candidate state columns: chunks of 256/256/512 (the last two
        # stage groups share one gather)
        CW = [CS, CS, 2 * CS]
        candC = [dram.tile([NT * 128 * CW[i]], BF16, name=f"candC{i}")
                 for i in range(3)]
        candCo = [dram.tile([NCORES * NT * 128 * CW[i]], BF16,
                            name=f"candCo{i}", addr_space="Shared")
                  for i in range(3)]
        # feature-major staging: y1 state rows whole, y2 state rows split
        # at 512 (batch 8); x-diffusion rows in their own small tiles
        y1t = [[dram.tile([SN, NOWN], BF16, name=f"y1_{g}{s}")
                for s in range(2)] for g in range(2)]
        y2tA = [[dram.tile([512, NOWN], BF16, name=f"y2A_{g}{s}")
                 for s in range(2)] for g in range(2)]
        y2tB = [[dram.tile([512, NOWN], BF16, name=f"y2B_{g}{s}")
                 for s in range(2)] for g in range(2)]
        y1xT = [dram.tile([XC, NOWN], BF16, name=f"y1x{s}")
                for s in range(2)]
        y2xT = [dram.tile([XC, NOWN], BF16, name=f"y2x{s}")
                for s in range(2)]
        candT_dram = dram.tile([BC, NOWN], BF16)
        rt_dram = dram.tile([B, H, NOWN], BF16)

        def agA_own(g, s, t):
            o = ((s * NT + t) * 128) * 512
            return agA[g].opt()[o:o + 128 * 512].rearrange(
                "(p f) -> p f", f=512)

        def agB_own(g, s, t):
            o = (t * 128) * 512
            return agB[g][s].opt()[o:o + 128 * 512].rearrange(
                "(p f) -> p f", f=512)

        def agX_own(s, t):
            o = ((s * NT + t) * 128) * XC
            return agX.opt()[o:o + 128 * XC].rearrange(
                "(p f) -> p f", f=XC)

        def outA_q(g, s, q):
            o = ((q * 2 + s) * NT * 128) * 512
            return agAo[g].opt()[o:o + NT * 128 * 512].rearrange(
                "(t p f) -> p t f", p=128, f=512)

        def outB_q(g, s, q):
            o = (q * NT * 128) * 512
            return agBo[g][s].opt()[o:o + NT * 128 * 512].rearrange(
                "(t p f) -> p t f", p=128, f=512)

        def outX_s(s):
            v = agXo.opt().rearrange("(q s2 t p f) -> s2 p q t f",
                                     s2=2, t=NT, p=128, f=XC)
            return v[s]

        def candC_own(i, t):
            o = t * 128 * CW[i]
            return candC[i].opt()[o:o + 128 * CW[i]].rearrange(
                "(p f) -> p f", f=CW[i])

        def candCo_q(i, q):
            o = q * NT * 128 * CW[i]
            return candCo[i].opt()[o:o + NT * 128 * CW[i]].rearrange(
                "(t p f) -> p t f", p=128, f=CW[i])

        def allgather(src, dst):
            nc.gpsimd.collective_compute(
                "AllGather", mybir.AluOpType.bypass, replica_groups=GROUP,
                ins=[src.opt()], outs=[dst.opt()])

        # ========== x-column diffusion hop 1 (once, both supports) =======
        def make_xpass():
            """y1x_s = A_s @ x: feature-major direct + node-major for the
            gather. Loads fire mid-first-sweep; the MMs run between the
            first main sweep and its staging (PSUM tiles allocated there
            so slot rotation can't deadlock against the sweep)."""
            pre = {}

            def loads():
                for q in range(NQ):
                    mx = sb_mov.tile([128, NT, XC], BF16, name=f"mx{q}",
                                     tag="movr", bufs=8)
                    src = d["x_nm"].ap().rearrange("(q t) p f -> q p t f",
                                                   t=NT)
                    nc.scalar.dma_start(mx[:], src[q])
                    pre[q] = mx

            ps_x = [None, None]

            def mms():
                for s in range(2):
                    ps_x[s] = psum.tile([XC, NOWN], F32, name=f"psx{s}",
                                        tag="acc", bufs=8)
                for q in range(NQ):
                    for tt in range(NT):
                        m = q * NT + tt
                        for s in range(2):
                            nc.tensor.matmul(ps_x[s][:], pre[q][:, tt, :],
                                             T_tile(s, m), start=(m == 0),
                                             stop=(m == MT - 1))

            def stage():
                for s in range(2):
                    xe = sb_ex.tile([XC, NOWN], BF16, name=f"xex{s}",
                                    tag="ragex", bufs=4)
                    nc.vector.tensor_copy(xe[:], ps_x[s][:])
                    nc.scalar.dma_start(y1xT[s].opt()[:, :], xe[:])
                    for t in range(NT):
                        tp = psum.tile([128, XC], BF16, name=f"xtp{s}",
                                       tag="acc", bufs=8)
                        nc.tensor.transpose(
                            tp[:], xe[:, t * 128:(t + 1) * 128],
                            identb[0:XC, 0:XC])
                        xnm = sb_sm.tile([128, XC], BF16, name=f"xnm{s}",
                                         tag="rnm", bufs=8)
                        nc.vector.tensor_copy(xnm[:], tp[:])
                        nc.scalar.dma_start(agX_own(s, t), xnm[:])
                allgather(agX, agXo)
            return loads, mms, stage

        # ========== x-column diffusion hop 2 (once, both supports) =======
        def emit_x2pass():
            for s in range(2):
                ps2 = psum.tile([XC, NOWN], F32, name=f"psx2{s}",
                                tag="acc", bufs=8)
                mrX = sb_mov.tile([128, NT, NQ, XC], BF16, name=f"mrX{s}",
                                  tag="mrX", bufs=2)
                vX = outX_s(s)
                for t in range(NT):
                    nc.sync.dma_start(mrX[:, t, :, :], vX[:, :, t, :])
                for q in range(NQ):
                    for tt in range(NT):
                        m = q * NT + tt
                        nc.tensor.matmul(ps2[:], mrX[:, tt, q, :],
                                         T_tile(s, m), start=(m == 0),
                                         stop=(m == MT - 1))
                xe = sb_ex.tile([XC, NOWN], BF16, name=f"x2ex{s}",
                                tag="ragex", bufs=4)
                nc.vector.tensor_copy(xe[:], ps2[:])
                nc.scalar.dma_start(y2xT[s].opt()[:, :], xe[:])

        # ============ hop-1 state sweeps + overlapped AG ============
        def emit_hop1_pair(pid, g, load_mov, y1_dst, stagger_T, xpass):
            """Y1_s[own rows, state cols] = A_s @ M for s in (0, 1). AG
            chunk A fires after the first sweep, B after the second.
            Feature-major y1 transposes are deferred (returned closure)."""
            kept = {}
            preloaded = {}
            xloads, xmms, xstage = xpass if xpass else (None, None, None)

            def emit_dyt(hh):
                # feature-major y1 staging for one sweep's outputs; h0's
                # runs between the sweeps (fills the AG-A window), h1's is
                # interleaved by the caller between the hop-2 A passes
                for s in range(2):
                    for j2 in range(2):
                        st4 = sb_sm.tile([128, 2, NOWN], BF16,
                                         name=f"st4{pid}", tag="st",
                                         bufs=4)
                        for jj2 in range(2):
                            j = j2 * 2 + jj2
                            for n in range(NT):
                                tp = psum.tile([128, 128], BF16,
                                               name=f"tp{pid}",
                                               tag="acc", bufs=8)
                                nc.tensor.transpose(
                                    tp[:],
                                    kept[(hh, s)][n][
                                        :, j * 128:(j + 1) * 128],
                                    identb[:])
                                nc.vector.tensor_copy(
                                    st4[:, jj2,
                                        n * 128:(n + 1) * 128], tp[:])
                        jj = hh * 4 + j2 * 2
                        nc.scalar.dma_start(
                            y1_dst[s].opt()[jj * 128:(jj + 2) * 128, :]
                            .rearrange("(j p) n -> p j n", j=2),
                            st4[:])
            for hh in range(2):
                ps_m = {}
                for s in range(2):
                    for n in range(NT):
                        ps_m[(s, n)] = psum.tile(
                            [128, 512], F32, name=f"psm{pid}_{hh}{s}{n}",
                            tag="acc", bufs=8)
                for q in range(NQ):
                    if stagger_T and hh == 0 and q + 2 < NQ:
                        load_Tch(q + 2)
                    if hh == 0 and q == 2 and xloads is not None:
                        xloads()
                    if (hh, q) in preloaded:
                        mv4 = preloaded.pop((hh, q))
                    else:
                        mv4 = sb_mov.tile([128, NT, 512], BF16,
                                          name=f"mv{pid}_{hh}_{q}",
                                          tag="mov", bufs=5)
                        load_mov(mv4, q, hh)
                    for tt in range(NT):
                        m = q * NT + tt
                        for s in range(2):
                            for n in range(NT):
                                nc.tensor.matmul(
                                    ps_m[(s, n)][:],
                                    T_tile(s, m)[:, n * 128:(n + 1) * 128],
                                    mv4[:, tt, :], start=(m == 0),
                                    stop=(m == MT - 1))
                if hh == 0:
                    # prefetch the next sweep's first tiles ahead of the
                    # staging stores
                    for qq in range(2):
                        mv4p = sb_mov.tile([128, NT, 512], BF16,
                                           name=f"mv{pid}_1_{qq}",
                                           tag="mov", bufs=5)
                        load_mov(mv4p, qq, 1)
                        preloaded[(1, qq)] = mv4p
                    # x-column matmuls bridge the PE across the AG-A
                    # window (their PSUM slots free after the first two
                    # extracts)
                    if xmms is not None:
                        xmms()
                for s in range(2):
                    exhs = []
                    for n in range(NT):
                        exh = sb_ex.tile([128, 512], BF16,
                                         name=f"ex{pid}{hh}{s}{n}",
                                         tag="ex", bufs=16)
                        nc.vector.tensor_copy(exh[:], ps_m[(s, n)][:])
                        dst = (agA_own(g, s, n) if hh == 0
                               else agB_own(g, s, n))
                        nc.scalar.dma_start(dst, exh[:])
                        exhs.append(exh)
                    kept[(hh, s)] = exhs
                    if hh == 1:
                        allgather(agB[g][s], agBo[g][s])
                if hh == 0:
                    allgather(agA[g], agAo[g])
                if hh == 0:
                    if xstage is not None:
                        xstage()
                    emit_dyt(0)

            return lambda: emit_dyt(1)

        # ======= hop-2 product: transposed form (feature-major out) =======
        def emit_hop2_pass(pid, s, g, part, yA_dst, yB_dst):
            ps = [psum.tile([128, NOWN], F32, name=f"ph2{pid}_{j}",
                            tag="acc", bufs=8) for j in range(4)]
            for q in range(NQ):
                mr4 = sb_mov.tile([128, NT, 512], BF16,
                                  name=f"mr{pid}_{q}", tag="mov", bufs=5)
                nc.sync.dma_start(
                    mr4[:], outA_q(g, s, q) if part == 0
                    else outB_q(g, s, q))
                for tt in range(NT):
                    m = q * NT + tt
                    for j in range(4):
                        nc.tensor.matmul(
                            ps[j][:],
                            mr4[:, tt, j * 128:(j + 1) * 128],
                            T_tile(s, m), start=(m == 0),
                            stop=(m == MT - 1))
            dst_t = yA_dst if part == 0 else yB_dst
            for j in range(4):
                exh = sb_ex.tile([128, NOWN], BF16, name=f"h2ex{pid}_{j}",
                                 tag="ex", bufs=16)
                nc.vector.tensor_copy(exh[:], ps[j][:])
                nc.scalar.dma_start(
                    dst_t.opt()[j * 128:(j + 1) * 128, :], exh[:])

        # ============== dense W stage (K-tiled, 4 batches/iter) ==========
        def load_ktiles(pi, block0_src, g):
            """Assemble the 330-feature contraction as 3 dense K-tiles of
            110 partitions. Block features are [state(64); x(2)] per
            batch: state rows come from the y1/y2 state tiles (y2 split
            A/B at batch 8, always batch-aligned), x rows from the shared
            x-diffusion tiles."""
            b0 = NB * pi
            kts = [sb_sm.tile([KT, NB, NOWN], BF16, name=f"kt{k}",
                              tag=f"kt{k}", bufs=3) for k in range(3)]
            for k, off, j, c, w in KT_RUNS:
                if j == 0:
                    nc.sync.dma_start(
                        kts[k][off:off + w, :, :],
                        block0_src[b0 * C:(b0 + NB) * C, :]
                        .rearrange("(b c) n -> c b n", b=NB)[c:c + w])
                    continue
                sidx = (j - 1) // 2 if j % 2 == 1 else j // 2 - 1
                # state part
                if c < H:
                    cs_hi = min(c + w, H)
                    if j % 2 == 1:
                        src = y1t[g][sidx].opt()
                        rlo = b0 * H
                    else:
                        src = (y2tA[g][sidx] if pi < 2
                               else y2tB[g][sidx]).opt()
                        rlo = b0 * H - (0 if pi < 2 else 512)
                    nc.sync.dma_start(
                        kts[k][off:off + cs_hi - c, :, :],
                        src[rlo:rlo + NB * H, :]
                        .rearrange("(b c) n -> c b n", b=NB)[c:cs_hi])
                # x part
                if c + w > H:
                    xsrc = (y1xT if j % 2 == 1 else y2xT)[sidx]
                    nc.sync.dma_start(
                        kts[k][off + max(0, H - c):off + w, :, :],
                        xsrc.opt()[b0 * DIN:(b0 + NB) * DIN, :]
                        .rearrange("(b c) n -> c b n",
                                   b=NB)[max(0, c - H):c + w - H])
            return kts

        def gate_mm(pi):
            b0 = NB * pi
            kts = load_ktiles(pi, d["xsT_own"].ap(), 0)
            zr_ps = [psum.tile([2 * H, NOWN], F32, name=f"zrps{b2}",
                               tag="acc", bufs=8) for b2 in range(NB)]
            for k in range(3):
                for b2 in range(NB):
                    nc.tensor.matmul(zr_ps[b2][:], wg_t[:, k, :],
                                     kts[k][:, b2, :],
                                     start=(k == 0), stop=(k == 2))
            zr4 = sb_sm.tile([2 * H, NB, NOWN], BF16, name="zr", tag="zr",
                             bufs=2)
            for b2 in range(NB):
                nc.scalar.activation(zr4[:, b2, :], zr_ps[b2][:],
                                     AF.Sigmoid, bias=bg_t[:])
            nc.scalar.dma_start(
                rt_dram.opt()[b0:b0 + NB].rearrange("b (h n) -> h b n",
                                                    h=H),
                zr4[H:2 * H, :, :])
            # kts[0][0:C] holds the direct input rows [state(64); x(2)]
            cT4 = sb_sm.tile([C, NB, NOWN], BF16, name="cT", tag="cT",
                             bufs=2)
            nc.vector.tensor_mul(cT4[0:H, :, :], zr4[0:H, :, :],
                                 kts[0][0:H, :, :])
            nc.vector.tensor_copy(cT4[H:C, :, :], kts[0][H:C, :, :])
            nc.scalar.dma_start(
                candT_dram.opt()[b0 * C:(b0 + NB) * C, :]
                .rearrange("(b c) n -> c b n", b=NB), cT4[:])
            return cT4

        def cand_stage(pi, cT4):
            # node-major candidate state chunk (batch-aligned, 256 cols);
            # chunks 2 and 3 share a buffer and gather once
            ci = min(pi, 2)
            co = (pi - 2) * CS if pi > 2 else 0
            for t in range(NT):
                ct4 = sb_sm.tile([128, NB, H], BF16, name="ctnm",
                                 tag="ctnm", bufs=4)
                for b2 in range(NB):
                    tp = psum.tile([128, H], BF16, name="ctps", tag="acc",
                                   bufs=8)
                    nc.tensor.transpose(
                        tp[:], cT4[0:H, b2, t * 128:(t + 1) * 128],
                        identb[0:H, 0:H])
                    nc.vector.tensor_copy(ct4[:, b2, :], tp[:])
                nc.scalar.dma_start(
                    candC_own(ci, t)[:, co:co + CS],
                    ct4[:].rearrange("p b c -> p (b c)"))
            if pi != 2:
                allgather(candC[ci], candCo[ci])

        def update_pi(pi):
            b0 = NB * pi
            kts = load_ktiles(pi, candT_dram.opt(), 1)
            hc_ps = [psum.tile([H, NOWN], F32, name=f"hcps{b2}", tag="acc",
                               bufs=8) for b2 in range(NB)]
            for k in range(3):
                for b2 in range(NB):
                    nc.tensor.matmul(hc_ps[b2][:], wu_t[:, k, :],
                                     kts[k][:, b2, :],
                                     start=(k == 0), stop=(k == 2))
            # out = hc + r * (state - hc); per-batch pipelined so the
            # trailing activation/DVE/store chain after the last matmul is
            # one batch deep, not four
            stT4 = sb_sm.tile([H, NB, NOWN], BF16, name="stTu", tag="stg",
                              bufs=2)
            nc.sync.dma_start(
                stT4[:],
                d["xsT_own"].ap()[b0 * C:(b0 + NB) * C, :]
                .rearrange("(b c) n -> c b n", b=NB)[0:H])
            rT4 = sb_sm.tile([H, NB, NOWN], BF16, name="rT", tag="rT",
                             bufs=2)
            nc.sync.dma_start(
                rT4[:],
                rt_dram.opt()[b0:b0 + NB].rearrange("b (h n) -> h b n",
                                                    h=H))
            hc4 = sb_sm.tile([H, NB, NOWN], BF16, name="hc", tag="zr",
                             bufs=2)
            tmp4 = sb_sm.tile([H, NB, NOWN], BF16, name="tmp", tag="tmp",
                              bufs=1)
            ot4 = sb_sm.tile([H, NB, NOWN], F32, name="ot", tag="ot",
                             bufs=1)
            for b2 in range(NB):
                nc.scalar.activation(hc4[:, b2, :], hc_ps[b2][:], AF.Tanh,
                                     bias=bu_t[:])
                nc.vector.tensor_sub(tmp4[:, b2, :], stT4[:, b2, :],
                                     hc4[:, b2, :])
                nc.vector.tensor_mul(tmp4[:, b2, :], rT4[:, b2, :],
                                     tmp4[:, b2, :])
                nc.vector.tensor_add(ot4[:, b2, :], hc4[:, b2, :],
                                     tmp4[:, b2, :])
                nc.scalar.dma_start(d["outT"].ap()[b0 + b2],
                                    ot4[:, b2, :])

        # ======================= GCN 1 (gate) =======================
        def g1_load_mov(t4, q, hh):
            src = d["st_nm"].ap().rearrange("(q t p) f -> q p t f",
                                            p=128, t=NT)
            nc.scalar.dma_start(t4[:],
                                src[q, :, :, hh * 512:(hh + 1) * 512])

        dyt1 = emit_hop1_pair("g1h1", 0, g1_load_mov, y1t[0], True,
                              make_xpass())
        dyt1()
        emit_hop2_pass("g1s0h2A", 0, 0, 0, y2tA[0][0], y2tB[0][0])
        emit_hop2_pass("g1s1h2A", 1, 0, 0, y2tA[0][1], y2tB[0][1])
        emit_x2pass()
        load_w_consts()
        ct0 = gate_mm(0)
        cand_stage(0, ct0)
        ct1 = gate_mm(1)
        cand_stage(1, ct1)
        emit_hop2_pass("g1s0h2B", 0, 0, 1, y2tA[0][0], y2tB[0][0])
        emit_hop2_pass("g1s1h2B", 1, 0, 1, y2tA[0][1], y2tB[0][1])
        ct2 = gate_mm(2)
        ct3 = gate_mm(3)
        cand_stage(2, ct2)
        cand_stage(3, ct3)

        # ======================= GCN 2 (update) =======================
        def g2_load_mov(t4, q, hh):
            if hh == 0:
                nc.sync.dma_start(t4[:, :, 0:CS], candCo_q(0, q))
                nc.sync.dma_start(t4[:, :, CS:512], candCo_q(1, q))
            else:
                nc.sync.dma_start(t4[:], candCo_q(2, q))

        dyt2 = emit_hop1_pair("g2h1", 1, g2_load_mov, y1t[1], False, None)
        dyt2()
        emit_hop2_pass("g2s0h2A", 0, 1, 0, y2tA[1][0], y2tB[1][0])
        emit_hop2_pass("g2s1h2A", 1, 1, 0, y2tA[1][1], y2tB[1][1])
        update_pi(0)
        update_pi(1)
        emit_hop2_pass("g2s0h2B", 0, 1, 1, y2tA[1][0], y2tB[1][0])
        emit_hop2_pass("g2s1h2B", 1, 1, 1, y2tA[1][1], y2tB[1][1])
        update_pi(2)
        update_pi(3)


def prepare_in_maps(x, state, support0, support1, W_gate, b_gate,
                    W_update, b_update):
    BFNP = mybir.dt.np(BF16)
    x_f = np.asarray(x, dtype=np.float32)
    state_f = np.asarray(state, dtype=np.float32)
    # node-major state (batch-major columns) and x columns
    st_nm = np.ascontiguousarray(
        state_f.transpose(1, 0, 2).reshape(N, SN)).astype(BFNP)
    x_nm = np.ascontiguousarray(
        x_f.transpose(1, 0, 2).reshape(N, XC)).astype(BFNP).reshape(
            MT, 128, XC)
    # feature-major input for W / elementwise uses [state(64); x(2)] rows
    sx_nm = np.ascontiguousarray(
        np.concatenate([state_f, x_f], axis=-1)
        .transpose(1, 0, 2).reshape(N, BC)).astype(np.float32)
    perm = np.r_[DIN:C, 0:DIN]                 # [x, state] -> [state, x]

    # fold the Chebyshev combination x2 = 2*A@x1 - x0 into W:
    # W0 -= (W2 + W4); W2 *= 2; W4 *= 2  (per 66-row block); then permute
    # every block's rows to the device feature order [state; x]
    def fold_perm(W):
        Wf = np.ascontiguousarray(W, dtype=np.float32).copy()
        Wf[0:C] -= Wf[2 * C:3 * C] + Wf[4 * C:5 * C]
        Wf[2 * C:3 * C] *= 2.0
        Wf[4 * C:5 * C] *= 2.0
        for j in range(5):
            Wf[j * C:(j + 1) * C] = Wf[j * C:(j + 1) * C][perm]
        return Wf.astype(BFNP)

    Wg_dev = fold_perm(W_gate)
    Wu_dev = fold_perm(W_update)

    bg = np.ascontiguousarray(b_gate, dtype=np.float32).reshape(2 * H, 1)
    bu = np.ascontiguousarray(b_update, dtype=np.float32).reshape(H, 1)
    s0b = np.asarray(support0, dtype=np.float32).astype(BFNP)
    s1b = np.asarray(support1, dtype=np.float32).astype(BFNP)

    in_maps = []
    for r in range(NCORES):
        n0 = r * NOWN
        in_maps.append({
            "Ts": np.ascontiguousarray(
                np.stack([s0b[n0:n0 + NOWN, :].T,
                          s1b[n0:n0 + NOWN, :].T])),
            "st_nm": st_nm,
            "x_nm": x_nm,
            "xsT_own": np.ascontiguousarray(
                sx_nm[n0:n0 + NOWN].T).astype(BFNP),
            "Wg": Wg_dev, "bg": bg, "Wu": Wu_dev, "bu": bu,
        })
    return in_maps


def assemble_output(results):
    out = np.empty((B, N, H), dtype=np.float32)
    for r in range(NCORES):
        n0 = r * NOWN
        out[:, n0:n0 + NOWN, :] = results[r]["outT"].transpose(0, 2, 1)
    return out


def get_nc():
    if "nc" not in _NC_CACHE:
        _NC_CACHE["nc"] = build_nc()
    return _NC_CACHE["nc"]


def kernel(x, state, support0, support1, W_gate, b_gate, W_update, b_update):
    nc = get_nc()
    in_maps = prepare_in_maps(x, state, support0, support1,
                              W_gate, b_gate, W_update, b_update)
    prev = os.environ.get("BASS_NEVER_TRACE")
    os.environ["BASS_NEVER_TRACE"] = "1"
    try:
        res = run_bass_kernel_spmd(nc, in_maps, list(range(NCORES)),
                                   trace=False)
    finally:
        if prev is None:
            os.environ.pop("BASS_NEVER_TRACE", None)
        else:
            os.environ["BASS_NEVER_TRACE"] = prev
    return assemble_output(res.results)
